# revision 1
# baseline (speedup 1.0000x reference)
"""Trainium2 Bass kernel for policy-masked attention (sparse_attention).

Shapes (hardcoded): x [4,1024,768], decision [4,768,2], qkv_w [2304,768],
qkv_b [2304], proj_w [768,768], proj_b [768], search_feat_len=768.

Sharding: 8 cores = 4 batches x 2 head-groups (6 heads each). Each core
computes its batch's q/k/v for its heads, the policy-masked softmax
(one-hot policy folded into the score matmul as 2 extra contraction rows
of -BIG * indicator outer products), attn @ v with a fused ones-column
producing the softmax denominator, and a partial output projection.
Host sums the two head-group partials per batch and adds proj_b.

Everything on-device runs in float32r (TF32-like, 11-bit mantissa) whose
products accumulate exactly in fp32 PSUM.
"""
import numpy as np

import concourse.bass as bass
import concourse.tile as tile
from concourse import bacc, mybir
from concourse.bass_utils import run_bass_kernel_spmd

F32 = mybir.dt.float32
F32R = mybir.dt.float32r
AF = mybir.ActivationFunctionType
ALU = mybir.AluOpType

B, N, C = 4, 1024, 768
H = 12
HD = 64
HPC = 6              # heads per core
KT = C // 128        # 6 contraction tiles
NT = N // 512        # 2 moving slices
MT = N // 128        # 8 key tiles
CO = C // 128        # 6 output-column tiles
SCALE = HD ** -0.5
BIG = 32768.0
EPS = 1e-6
N_CORES = 8


def round_fp32r(a: np.ndarray) -> np.ndarray:
    """Round-to-nearest-even to 11-bit mantissa (the PE fp32r format)."""
    bits = np.ascontiguousarray(a, dtype=np.float32).view(np.uint32)
    r = bits + np.uint32(0x7FF) + ((bits >> np.uint32(12)) & np.uint32(1))
    r &= np.uint32(0xFFFFF000)
    return r.view(np.float32)


def _body(nc, tc, t, with_vbias=True):
    """Emit one full forward pass. t = dict of dram tensor handles."""
    import contextlib
    with contextlib.ExitStack() as ctx:
        consts = ctx.enter_context(tc.tile_pool(name="consts", bufs=1))
        headp = ctx.enter_context(tc.tile_pool(name="headp", bufs=1))

        xT_sb = consts.tile([128, KT, N], F32R)
        wqkT_sb = consts.tile([128, KT, C], F32R)
        wvT_sb = consts.tile([128, KT, HPC * HD], F32R)
        vbias_sb = consts.tile([1, HPC * HD], F32R)
        ones1_sb = consts.tile([1, 128], F32R)
        qkb_sb = consts.tile([128, KT], F32)
        projT_sb = consts.tile([128, 3, C], F32R)
        pbias_sb = consts.tile([128, CO], F32)

        # Bulk x/w chunks head the sync queue (they gate the first matmuls);
        # small early-needed tiles ride the scalar engine's queue; projT is
        # not read until phase 3, so it goes last on sync.
        for kt in range(KT):
            nc.sync.dma_start(out=xT_sb[:, kt, :], in_=t["xT"].ap()[:, kt, :])
            nc.sync.dma_start(out=wqkT_sb[:, kt, :], in_=t["wqkT"].ap()[:, kt, :])
        nc.sync.dma_start(out=wvT_sb, in_=t["wvT"].ap())
        nc.sync.dma_start(out=projT_sb, in_=t["projT"].ap())
        nc.scalar.dma_start(out=qkb_sb, in_=t["qkb"].ap())
        nc.scalar.dma_start(out=vbias_sb, in_=t["vbias"].ap())
        nc.scalar.dma_start(out=ones1_sb, in_=t["ones1"].ap())
        nc.scalar.dma_start(out=pbias_sb, in_=t["pbias"].ap())

        # Per-head q/k tiles [66, N]: rows 0-63 head data, rows 64-65 the
        # rank-2 log-mask factors (k side: -BIG*p0,-BIG*p1; q side: p1,p0).
        qh = [headp.tile([66, N], F32R, name=f"qh{h}", tag=f"qh{h}")
              for h in range(HPC)]
        kh = [headp.tile([66, N], F32R, name=f"kh{h}", tag=f"kh{h}")
              for h in range(HPC)]
        for h in range(HPC):
            nc.scalar.dma_start(out=qh[h][64:66, :], in_=t["mq"].ap())
            nc.scalar.dma_start(out=kh[h][64:66, :], in_=t["mk"].ap())

        # V in token-major layout with a fused ones column: [128, MT, 6*65]
        V_sb = consts.tile([128, MT, HPC * 65], F32R)
        vv = V_sb.rearrange("p m (h e) -> p m h e", h=HPC)
        nc.sync.dma_start(out=vv[:, :, :, 64:65],
                          in_=t["vones"].ap().rearrange(
                              "p (m h e) -> p m h e", m=MT, h=HPC))

        warm = consts.tile([1, 1], F32)
        nc.scalar.activation(warm, qkb_sb[0:1, 0:1], AF.Exp)

        # ---- Phase 1a: q,k generation (qkvT = Wqk @ x^T) ----
        with tc.tile_pool(name="ps1", bufs=2, space="PSUM") as ps1:
            for j in range(KT):
                tiles = qh if j < 3 else kh
                jj = j % 3
                ps = ps1.tile([128, N], F32, tag="qk", name=f"qkps{j}")
                for n in range(NT):
                    for kt in range(KT):
                        nc.tensor.matmul(ps[:, n * 512:(n + 1) * 512],
                                         lhsT=wqkT_sb[:, kt, j * 128:(j + 1) * 128],
                                         rhs=xT_sb[:, kt, n * 512:(n + 1) * 512],
                                         start=(kt == 0), stop=(kt == KT - 1))
                for half in range(2):
                    h = 2 * jj + half
                    nc.vector.tensor_scalar(
                        out=tiles[h][0:64, :],
                        in0=ps[half * 64:(half + 1) * 64, :],
                        scalar1=qkb_sb[half * 64:(half + 1) * 64, j:j + 1],
                        scalar2=None, op0=ALU.add)

            # ---- Phase 1b: V in token-major layout (+ bias via ones row) ----
            for m in range(MT):
                psv = ps1.tile([128, HPC * HD], F32, tag="v")
                for kt in range(KT):
                    nc.tensor.matmul(psv,
                                     lhsT=xT_sb[:, kt, m * 128:(m + 1) * 128],
                                     rhs=wvT_sb[:, kt, :],
                                     start=(kt == 0),
                                     stop=(not with_vbias and kt == KT - 1))
                if with_vbias:
                    nc.tensor.matmul(psv, lhsT=ones1_sb, rhs=vbias_sb,
                                     start=False, stop=True)
                nc.vector.tensor_copy(vv[:, m, :, 0:64],
                                      psv.rearrange("p (h d) -> p h d", h=HPC))

        # ---- Phase 2: per-head masked scores, exp, AV(+Z) ----
        zpool = ctx.enter_context(tc.tile_pool(name="zpool", bufs=1))
        zst = zpool.tile([1, HPC, NT, 512], F32)   # Z rows (+EPS), partition 0
        zbp = zpool.tile([128, 3, N], F32)         # 1/Z broadcast per pair
        Ab = zpool.tile([128, 3, N], F32R)
        zdram = ctx.enter_context(tc.tile_pool(name="zdram", bufs=1,
                                               space="DRAM"))
        zd = zdram.tile([3, 2, N], F32)

        Ust = zpool.tile([128, 3, N], F32)

        with tc.tile_pool(name="stp", bufs=2, space="PSUM") as stp, \
             tc.tile_pool(name="up", bufs=2, space="PSUM") as up, \
             tc.tile_pool(name="ep", bufs=6) as ep, \
             tc.tile_pool(name="zwp", bufs=2) as zwp:
            for h in range(HPC):
                ups = up.tile([65, N], F32, name=f"u{h}", tag="u")
                for m in range(MT):
                    st = stp.tile([128, N], F32, tag="st")
                    for n in range(NT):
                        nc.tensor.matmul(st[:, n * 512:(n + 1) * 512],
                                         lhsT=kh[h][:, m * 128:(m + 1) * 128],
                                         rhs=qh[h][:, n * 512:(n + 1) * 512],
                                         start=True, stop=True)
                    e = ep.tile([128, N], F32R, tag="e")
                    nc.scalar.activation(e, st, AF.Exp)
                    for n in range(NT):
                        nc.tensor.matmul(ups[:, n * 512:(n + 1) * 512],
                                         lhsT=V_sb[:, m, h * 65:(h + 1) * 65],
                                         rhs=e[:, n * 512:(n + 1) * 512],
                                         start=(m == 0), stop=(m == MT - 1))
                tt, half = h // 2, h % 2
                nc.vector.tensor_scalar(
                    out=zst[0:1, h, :, :],
                    in0=ups[64:65, :].rearrange("p (n f) -> p n f", n=NT),
                    scalar1=EPS, scalar2=None, op0=ALU.add)
                nc.vector.tensor_copy(Ust[half * 64:(half + 1) * 64, tt, :],
                                      ups[0:64, :])

                if half == 1:
                    # Z chain for this head pair: reciprocal on 64 lanes,
                    # DRAM bounce, partition-broadcast, normalize. Overlaps
                    # with the next pair's score/AV compute.
                    zw = zwp.tile([64, 2 * N // 64], F32, tag="zw")
                    nc.sync.dma_start(out=zw, in_=zst[0:1, h - 1:h + 1, :, :])
                    nc.vector.reciprocal(zw, zw)
                    nc.sync.dma_start(out=zd[tt], in_=zw)
                    src = zd[tt]
                    bsrc = bass.AP(tensor=src.tensor, offset=src.offset,
                                   ap=[src.ap[0], [0, 64], src.ap[1]])
                    nc.sync.dma_start(out=zbp[:, tt, :], in_=bsrc)
                    nc.vector.tensor_mul(Ab[:, tt, :], Ust[:, tt, :],
                                         zbp[:, tt, :])

        # ---- Phase 3: output projection (partial, this head-group) ----
        with tc.tile_pool(name="pj", bufs=2, space="PSUM") as pj, \
             tc.tile_pool(name="op", bufs=3) as op:
            for co in range(CO):
                ps = pj.tile([128, N], F32, tag="pj", name=f"pjps{co}")
                for n in range(NT):
                    for kt in range(3):
                        nc.tensor.matmul(ps[:, n * 512:(n + 1) * 512],
                                         lhsT=projT_sb[:, kt, co * 128:(co + 1) * 128],
                                         rhs=Ab[:, kt, n * 512:(n + 1) * 512],
                                         start=(kt == 0), stop=(kt == 2))
                ot = op.tile([128, N], F32, tag="o")
                eng = nc.vector if co % 2 == 0 else nc.scalar
                if co % 2 == 0:
                    nc.vector.tensor_scalar(out=ot, in0=ps,
                                            scalar1=pbias_sb[:, co:co + 1],
                                            scalar2=None, op0=ALU.add)
                else:
                    nc.scalar.activation(ot, ps, AF.Identity,
                                         bias=pbias_sb[:, co:co + 1], scale=1.0)
                nc.sync.dma_start(out=t["outT"].ap()[:, co, :], in_=ot)


_NC_CACHE = {}


def build_nc(reps: int = 1, with_vbias: bool = True, loop: int = 0):
    key = (reps, with_vbias, loop)
    if key in _NC_CACHE:
        return _NC_CACHE[key]
    nc = bacc.Bacc("TRN2", target_bir_lowering=False, debug=False,
                   num_devices=N_CORES)
    t = {
        "xT": nc.dram_tensor("xT", [128, KT, N], F32R, kind="ExternalInput"),
        "wqkT": nc.dram_tensor("wqkT", [128, KT, C], F32R, kind="ExternalInput"),
        "qkb": nc.dram_tensor("qkb", [128, KT], F32, kind="ExternalInput"),
        "wvT": nc.dram_tensor("wvT", [128, KT, HPC * HD], F32R,
                              kind="ExternalInput"),
        "vbias": nc.dram_tensor("vbias", [1, HPC * HD], F32R,
                                kind="ExternalInput"),
        "ones1": nc.dram_tensor("ones1", [1, 128], F32R, kind="ExternalInput"),
        "mq": nc.dram_tensor("mq", [2, N], F32R, kind="ExternalInput"),
        "mk": nc.dram_tensor("mk", [2, N], F32R, kind="ExternalInput"),
        "projT": nc.dram_tensor("projT", [128, 3, C], F32R,
                                kind="ExternalInput"),
        "pbias": nc.dram_tensor("pbias", [128, CO], F32, kind="ExternalInput"),
        "vones": nc.dram_tensor("vones", [128, MT * HPC], F32R,
                                kind="ExternalInput"),
        "outT": nc.dram_tensor("outT", [128, CO, N], F32,
                               kind="ExternalOutput"),
    }
    with tile.TileContext(nc) as tc:
        if loop:
            with tc.For_i(0, loop, 1):
                _body(nc, tc, t, with_vbias=with_vbias)
        else:
            for _ in range(reps):
                _body(nc, tc, t, with_vbias=with_vbias)
    nc.compile()
    _NC_CACHE[key] = nc
    return nc


def _is_onehot(decision: np.ndarray) -> bool:
    vals_ok = np.all((decision == 0.0) | (decision == 1.0))
    return bool(vals_ok and np.all(decision.sum(-1) == 1.0))


def _policy_vectors(decision_b: np.ndarray, S: int):
    p0 = np.zeros(N, np.float32)
    p0[:N - S] = 1.0
    p1 = np.zeros(N, np.float32)
    p1[N - S:] = decision_b[:, 0]
    p2 = np.zeros(N, np.float32)
    p2[N - S:] = decision_b[:, 1]
    return p0, p1, p2


def make_in_maps(x, decision, qkv_w, qkv_b, proj_w, proj_b, S):
    in_maps = []
    xT_cache = {}
    ones1 = round_fp32r(np.ones((1, 128), np.float32))
    vones = np.ones((128, MT * HPC), np.float32)
    for core in range(N_CORES):
        b, hg = core // 2, core % 2
        if b not in xT_cache:
            xT = np.ascontiguousarray(x[b].T)  # [C, N]
            xT_cache[b] = round_fp32r(
                xT.reshape(KT, 128, N).transpose(1, 0, 2))
        qs = slice(hg * 384, hg * 384 + 384)
        ks = slice(C + hg * 384, C + hg * 384 + 384)
        vs = slice(2 * C + hg * 384, 2 * C + hg * 384 + 384)
        Wqk = np.concatenate([qkv_w[qs] * SCALE, qkv_w[ks]], axis=0)  # [768,C]
        wqkT = round_fp32r(
            Wqk.T.reshape(KT, 128, C).transpose(1, 0, 2))
        bqk = np.concatenate([qkv_b[qs] * SCALE, qkv_b[ks]])
        qkb = np.ascontiguousarray(bqk.reshape(KT, 128).T, dtype=np.float32)
        wvT = round_fp32r(
            qkv_w[vs].T.reshape(KT, 128, 384).transpose(1, 0, 2))
        vbias = round_fp32r(qkv_b[vs].reshape(1, 384))
        p0, p1, p2 = _policy_vectors(decision[b], S)
        mq = round_fp32r(np.stack([p1, p0]))
        mk = round_fp32r(np.stack([-BIG * p0, -BIG * p1]))
        projT = round_fp32r(
            proj_w[:, hg * 384:hg * 384 + 384].T
            .reshape(3, 128, C).transpose(1, 0, 2))
        if hg == 0:
            pbias = np.ascontiguousarray(
                proj_b.reshape(CO, 128).T, dtype=np.float32)
        else:
            pbias = np.zeros((128, CO), np.float32)
        in_maps.append({
            "xT": xT_cache[b], "wqkT": wqkT, "qkb": qkb, "wvT": wvT,
            "vbias": vbias, "ones1": ones1, "mq": mq, "mk": mk,
            "projT": projT, "pbias": pbias, "vones": vones,
        })
    return in_maps


def _numpy_fallback(x, decision, qkv_w, qkv_b, proj_w, proj_b, S):
    """Direct port of the reference for non-one-hot policies."""
    out = np.empty((B, N, C), np.float32)
    for b in range(B):
        p0, p1, p2 = _policy_vectors(decision[b], S)
        qkv = x[b] @ qkv_w.T + qkv_b
        qkv = qkv.reshape(N, 3, H, HD).transpose(1, 2, 0, 3)
        q, k, v = qkv[0], qkv[1], qkv[2]          # [H, N, HD]
        s = p0 + p1 + p2
        ap = (np.outer(s, s) - np.outer(p0, p1) - np.outer(p1, p0))
        ap = ap + (1.0 - ap) * np.eye(N, dtype=np.float32)
        attn = np.einsum('hnd,hmd->hnm', q, k).astype(np.float32) * SCALE
        m = attn.max(-1, keepdims=True)
        e = np.exp(attn - m) * ap[None]
        p = (e + EPS / N) / (e.sum(-1, keepdims=True) + EPS)
        o = np.einsum('hnm,hmd->hnd', p, v)
        out[b] = o.transpose(1, 0, 2).reshape(N, C) @ proj_w.T + proj_b
    return out


def kernel(x, decision, qkv_w, qkv_b, proj_w, proj_b, search_feat_len):
    x = np.asarray(x, np.float32)
    decision = np.asarray(decision, np.float32)
    qkv_w = np.asarray(qkv_w, np.float32)
    qkv_b = np.asarray(qkv_b, np.float32)
    proj_w = np.asarray(proj_w, np.float32)
    proj_b = np.asarray(proj_b, np.float32)
    S = int(np.asarray(search_feat_len))

    if not _is_onehot(decision):
        return _numpy_fallback(x, decision, qkv_w, qkv_b, proj_w, proj_b, S)

    nc = build_nc(with_vbias=bool(np.any(qkv_b[2 * C:] != 0.0)))
    in_maps = make_in_maps(x, decision, qkv_w, qkv_b, proj_w, proj_b, S)
    res = run_bass_kernel_spmd(nc, in_maps, core_ids=list(range(N_CORES)))

    out = np.empty((B, N, C), np.float32)
    for b in range(B):
        partial = res.results[2 * b]["outT"] + res.results[2 * b + 1]["outT"]
        out[b] = partial.transpose(1, 0, 2).reshape(C, N).T
    return out

